# revision 1
# baseline (speedup 1.0000x reference)
"""Trainium2 Bass kernel for nn_DampedIMEX1Layer.

Math: the reference's per-step transition matrix M (2x2 per diagonal state p)
is CONSTANT over time (G_seq rows are identical), so the associative scan is a
constant-coefficient linear recurrence
    z_t = M z_{t-1} + c * u_t,   u = x @ Bc^T (complex),  y_t = z_t[1]
and the output is out = Re(ys @ Cc^T) + D*x.

Device algorithm (per core = one batch element, data-parallel over 8 cores),
with chunk size T=8 over L=8192 (C=1024 chunks):
  local part   out_loc^T[:, iT+tau] = sum_{s<=tau} Phi_s @ x^T[:, iT+tau-s]
               with Phi_s = Re(Cc diag(K_s) Bc) (+ diag(D) at s=0) folded on
               host; computed as tau-major f32r lag matmuls on PE.
  block ends   g_i = sum_j M^{T-1-j} c u_{iT+j}: 4 real streams via PE
               matmuls with weights diag(w) B folded on host.
  carry chain  S_i = M^T S_{i-1} + g_i: blocked scalar_tensor_tensor scan on
               DVE (block 16, two nested levels), fp32.
  injection    out^T[:, iT+tau] += Re(Cc M^{tau+1} S_{i-1}): tau-major PE
               matmuls with weights Cc-side-folded on host.
All matmuls run as float32r (TF32) on host-pre-rounded data; accumulation is
fp32 in PSUM, so the only precision loss is the one-time TF32 input rounding.
"""
import numpy as np

import concourse.bacc as bacc
import concourse.mybir as mybir
from concourse.tile import TileContext
from concourse import bass_utils

P = 128
H = 128
L = 8192
BSZ = 8
T = 8                  # chunk length (timesteps)
C = L // T             # 1024 chunks
CSEG = 512             # chunks per psum segment
NSEG = C // CSEG       # 2
Q1 = 8                 # chain level-1 block length (chunks)
B1 = C // Q1           # 128 level-1 blocks
Q2 = 16                # chain level-2 block length (level-1 blocks)
B2 = B1 // Q2          # 8 level-2 blocks

F32 = mybir.dt.float32
F32R = mybir.dt.float32r


def _tf32(a):
    """Round fp32 array to TF32 (10-bit mantissa, round-to-nearest)."""
    a = np.ascontiguousarray(a, dtype=np.float32)
    b = a.view(np.uint32).astype(np.uint64)
    b = ((b + 0x1000) & 0xFFFFE000) & 0xFFFFFFFF
    return b.astype(np.uint32).view(np.float32)


def _host_params(A_diag, G_diag, dt):
    """Reproduce reference's projections in fp32, return fp64 M (P,2,2), c (P,2)."""
    f = np.float32
    dt_s = (1.0 / (1.0 + np.exp(-dt.astype(np.float64)))).astype(f)
    A = np.maximum(A_diag.astype(f), f(0.0))
    G = np.maximum(G_diag.astype(f), f(0.0))
    dt2 = np.maximum(dt_s * dt_s, f(1e-6))
    s = np.sqrt(f(1.0) + dt_s * G)
    A_low = (f(2.0) + dt_s * G - f(2.0) * s) / dt2
    A_high = (f(2.0) + dt_s * G + f(2.0) * s) / dt2
    A_fin = A_low + np.maximum(A - A_low, f(0)) - np.maximum(A - A_high, f(0))
    S = f(1.0) + dt_s * G
    M11 = f(1.0) / S
    M12 = -(dt_s / S) * A_fin
    M21 = dt_s / S
    M22 = f(1.0) - (dt_s * dt_s / S) * A_fin
    c1 = dt_s / S
    c2 = dt_s * dt_s / S
    M = np.stack([np.stack([M11, M12], -1), np.stack([M21, M22], -1)], -2)
    c = np.stack([c1, c2], -1)
    return M.astype(np.float64), c.astype(np.float64)


def _mat_powers(M, n):
    """(n, P, 2, 2): M^0 .. M^{n-1}."""
    out = np.empty((n, P, 2, 2))
    out[0] = np.eye(2)[None]
    for i in range(1, n):
        out[i] = np.einsum('pij,pjk->pik', M, out[i - 1])
    return out


def _host_weights(A_diag, G_diag, dt, B, C_, D):
    """Build all device weight matrices / coefficient vectors on host."""
    M, c = _host_params(A_diag, G_diag, dt)
    Bre = B[..., 0].astype(np.float64)   # (P,H)
    Bim = B[..., 1].astype(np.float64)
    Cre = C_[..., 0].astype(np.float64)  # (H,P)
    Cim = C_[..., 1].astype(np.float64)

    Mp = _mat_powers(M, T + 1)                    # M^0..M^T
    K = np.einsum('spij,pj->spi', Mp[:T], c)[:, :, 1]   # (T,P) real
    MT = Mp[T]                                    # (P,2,2)

    # Phi_s^T as lhsT: lhsT[h', h] = Phi_s[h, h']
    # Phi_s = Cre diag(K_s) Bre - Cim diag(K_s) Bim   (H,H)
    PhiT = np.empty((T, H, H), np.float32)
    for s in range(T):
        Phi = (Cre * K[s]) @ Bre - (Cim * K[s]) @ Bim
        if s == 0:
            Phi = Phi + np.diag(D.astype(np.float64))
        PhiT[s] = _tf32(Phi.T.astype(np.float32))

    # extraction: w_j = M^{T-1-j} c  (T,P,2); V^{j,k,str} = diag(w_j[:,k]) Bstr
    # lhsT = V^T = (H,P): Bstr.T * w
    w = np.einsum('spij,pj->spi', Mp[:T][::-1], c)    # w[j] = M^{T-1-j} c
    VT = np.empty((T, 2, 2, H, P), np.float32)        # [j, kappa, str]
    for j in range(T):
        for k in range(2):
            VT[j, k, 0] = _tf32((Bre.T * w[j, :, k]).astype(np.float32))
            VT[j, k, 1] = _tf32((Bim.T * w[j, :, k]).astype(np.float32))

    # injection: Winj[tau] = row 1 of M^{tau+1}: (T,P,2)
    # Psi^{tau,k,re} = Cre diag(Winj[tau,:,k]); Psi^{tau,k,im} = -Cim diag(...)
    # lhsT = Psi^T = diag(Winj) @ Cstr^T  (P,H)
    Winj = Mp[1:T + 1][:, :, 1, :]                    # (T,P,2)
    PsiT = np.empty((T, 2, 2, P, H), np.float32)
    for t in range(T):
        for k in range(2):
            PsiT[t, k, 0] = _tf32((Cre.T * 1.0 * Winj[t, :, k][:, None]).astype(np.float32))
            PsiT[t, k, 1] = _tf32((-Cim.T * Winj[t, :, k][:, None]).astype(np.float32))

    # chain coefficient packs (fp32, not rounded - DVE fp32 math)
    def flat22(A):   # (P,2,2) -> (4,P): [a00,a01,a10,a11]
        return np.stack([A[:, 0, 0], A[:, 0, 1], A[:, 1, 0], A[:, 1, 1]], 0).astype(np.float32)

    MTp = _mat_powers(MT, Q1 + 1)                 # MT^0..MT^Q1
    MT16 = MTp[Q1]                                # MT^16
    MT16p = _mat_powers(MT16, Q2 + 1)
    MT128 = MT16p[Q2]

    chain_s1 = flat22(MT).T                       # (P,4)
    chain_bc = flat22(MT16).T                     # (P,4)
    chain_bcA = flat22(MT128).T                   # (P,4)
    fix_bc = np.concatenate([flat22(MT16p[t2 + 1]) for t2 in range(Q2)], 0).T  # (P,Q2*4)
    fix_L1 = np.concatenate([flat22(MTp[t + 1]) for t in range(Q1)], 0).T      # (P,Q1*4)

    # seeds for on-device weight build: [Bre | Bim | CreT | CimT | diagD]
    seed = np.concatenate(
        [Bre, Bim, Cre.T, Cim.T, np.diag(D.astype(np.float64)), np.eye(P)], axis=1)
    # scale vectors: [Ks(T) | -Ks(T) | vw(2T: j,k) | pw(2T: tau,k) | -pw(2T)]
    vw = np.stack([w[j, :, k] for j in range(T) for k in range(2)], 1)    # (P,2T)
    pw = np.stack([Winj[t, :, k] for t in range(T) for k in range(2)], 1)
    kw = np.concatenate([K.T, -K.T, vw, pw, -pw], axis=1)                 # (P,8T)
    coef = np.concatenate([chain_s1, chain_bc, chain_bcA, fix_bc, fix_L1], axis=1)
    return dict(seed=_tf32(seed.astype(np.float32)),
                kw=np.ascontiguousarray(kw.astype(np.float32)),
                coef=np.ascontiguousarray(coef.astype(np.float32)))


def _build_nc():
    nc = bacc.Bacc("TRN2", target_bir_lowering=False, debug=False, num_devices=8)
    AluOp = mybir.AluOpType

    NCOEF = 4 + 4 + 4 + Q2 * 4 + Q1 * 4
    x_d = nc.dram_tensor("x", (P, L), F32R, kind="ExternalInput").ap()
    seed_d = nc.dram_tensor("seed", (P, 6 * H), F32R, kind="ExternalInput").ap()
    kw_d = nc.dram_tensor("kw", (P, 8 * T), F32, kind="ExternalInput").ap()
    coef_d = nc.dram_tensor("coef", (P, NCOEF), F32, kind="ExternalInput").ap()
    out_d = nc.dram_tensor("out", (P, L), F32, kind="ExternalOutput").ap()

    with TileContext(nc) as tc:
        with (
            tc.tile_pool(name="const", bufs=1) as cp,
            tc.tile_pool(name="state", bufs=1) as sp,
            tc.tile_pool(name="psg", bufs=2, space="PSUM") as psg,
            tc.tile_pool(name="pso", bufs=2, space="PSUM") as pso,
        ):
            # ---------- load inputs ----------
            seed_sb = cp.tile([P, 6 * H], F32R, tag="seed")
            nc.sync.dma_start(seed_sb[:], seed_d)
            kw_sb = cp.tile([P, 8 * T], F32, tag="kw")
            nc.sync.dma_start(kw_sb[:], kw_d)
            coef_sb = cp.tile([P, NCOEF], F32, tag="coef")
            nc.sync.dma_start(coef_sb[:], coef_d)
            # x in 4 slices so downstream work can start early
            x_sb = cp.tile([P, L], F32R, tag="x")
            XSL = L // 4
            for i in range(4):
                nc.sync.dma_start(x_sb[:, i * XSL:(i + 1) * XSL],
                                  x_d[:, i * XSL:(i + 1) * XSL])

            bre = seed_sb[:, 0:H]
            bim = seed_sb[:, H:2 * H]
            cret = seed_sb[:, 2 * H:3 * H]
            cimt = seed_sb[:, 3 * H:4 * H]
            diagd = seed_sb[:, 4 * H:5 * H]

            # ---------- on-device weight build ----------
            ident = seed_sb[:, 5 * H:6 * H]

            phi_sb = cp.tile([H, T * P], F32R, tag="phi")
            v_sb = cp.tile([H, T * 4 * P], F32R, tag="v")
            psi_sb = cp.tile([P, T * 4 * H], F32R, tag="psi")
            AluB = mybir.AluOpType

            # V^T slots: TS scale of B by vw -> PE transpose -> ACT copy
            for j in range(T):
                for k in range(2):
                    vwcol = kw_sb[:, 2 * T + j * 2 + k:2 * T + j * 2 + k + 1]
                    for ri, bsrc in ((0, bre), (1, bim)):
                        vtmp = sp.tile([P, H], F32R, tag="vtmp")
                        nc.vector.tensor_scalar(vtmp[:], bsrc, vwcol, None, AluB.mult)
                        ptr = pso.tile([P, H], F32R, tag="ptr")
                        nc.tensor.transpose(ptr[:], vtmp[:], ident[:])
                        qidx = (j * 2 + k) * 2 + ri
                        nc.scalar.copy(v_sb[:, qidx * P:(qidx + 1) * P], ptr[:])

            # Psi^T slots: TS scale of CreT / CimT by pw / -pw
            for tau in range(T):
                for k in range(2):
                    pwcol = kw_sb[:, 4 * T + tau * 2 + k:4 * T + tau * 2 + k + 1]
                    npwcol = kw_sb[:, 6 * T + tau * 2 + k:6 * T + tau * 2 + k + 1]
                    qre = (tau * 2 + k) * 2 + 0
                    qim = (tau * 2 + k) * 2 + 1
                    nc.vector.tensor_scalar(
                        psi_sb[:, qre * H:(qre + 1) * H], cret, pwcol, None, AluB.mult)
                    nc.vector.tensor_scalar(
                        psi_sb[:, qim * H:(qim + 1) * H], cimt, npwcol, None, AluB.mult)

            x3 = x_sb[:].rearrange("p (c t) -> p c t", t=T)

            # ---------- block-end extraction (PE) ----------
            # g streams q = 2*kappa + str; stored re/im interleaved by chunk:
            # gk tile (P, 2C): col 2i+str_is_im? -> layout: comp kappa tile with
            # col = 2*i + ri where ri in {0(re),1(im)}.
            g0 = sp.tile([P, 2 * C], F32, tag="g0")
            g1 = sp.tile([P, 2 * C], F32, tag="g1")
            gt = {0: g0, 1: g1}
            for cs in range(NSEG):
                for k in range(2):
                    for ri in range(2):
                        ps = psg.tile([P, CSEG], F32, tag="psg")
                        for j in range(T):
                            qidx = (j * 2 + k) * 2 + ri
                            nc.tensor.matmul(
                                ps[:],
                                v_sb[:, qidx * P:(qidx + 1) * P],
                                x3[:, cs * CSEG:(cs + 1) * CSEG, j],
                                start=(j == 0), stop=(j == T - 1),
                            )
                        # strided interleave copy into g tile
                        dst = gt[k][:].rearrange("p (c r) -> p c r", r=2)[
                            :, cs * CSEG:(cs + 1) * CSEG, ri]
                        nc.vector.tensor_copy(dst, ps[:])

            # Phi^T slots: rhs = (+-Ks) * C*T; lhsT = B natural; + diagD @ I at s=0
            for s in range(T):
                t1 = sp.tile([P, H], F32R, tag="t1")
                t2 = sp.tile([P, H], F32R, tag="t2")
                nc.vector.tensor_scalar(t1[:], cret, kw_sb[:, s:s + 1], None, AluB.mult)
                nc.vector.tensor_scalar(t2[:], cimt, kw_sb[:, T + s:T + s + 1], None, AluB.mult)
                pph = pso.tile([H, P], F32, tag="ptr")
                nc.tensor.matmul(pph[:], bre, t1[:], start=True, stop=False)
                nc.tensor.matmul(pph[:], bim, t2[:], start=False, stop=(s != 0))
                if s == 0:
                    nc.tensor.matmul(pph[:], diagd, ident[:], start=False, stop=True)
                nc.scalar.copy(phi_sb[:, s * P:(s + 1) * P], pph[:])
            cs1 = coef_sb[:, 0:4]
            cbc = coef_sb[:, 4:8]
            cbcA = coef_sb[:, 8:12]
            fbc = coef_sb[:, 12:12 + Q2 * 4]
            fL1 = coef_sb[:, 12 + Q2 * 4:12 + Q2 * 4 + Q1 * 4]

            # ---------- carry chain (DVE, fp32), pipelined by halves ----------
            s0 = g0  # stage-1 computes states in place over g
            s1 = g1
            HB = B1 // NSEG          # blocks per half (64)
            HA = B2 // NSEG          # A-blocks per half (4)

            def blkslice(tile, t, blo, bhi):
                return tile[:].rearrange("p (b t r) -> p b t r", t=Q1, r=2)[
                    :, blo:bhi, t, :]

            bc0 = sp.tile([P, B1 * 2], F32, tag="bc0")
            bc1 = sp.tile([P, B1 * 2], F32, tag="bc1")

            def bslice(tile, t2, alo, ahi):
                return tile[:].rearrange("p (a t r) -> p a t r", t=Q2, r=2)[
                    :, alo:ahi, t2, :]

            def stage1f():
                blo, bhi = 0, B1
                for t in range(1, Q1):
                    a0 = blkslice(s0, t - 1, blo, bhi)
                    a1 = blkslice(s1, t - 1, blo, bhi)
                    tm0 = sp.tile([P, bhi - blo, 2], F32, tag="tm0")
                    tm1 = sp.tile([P, bhi - blo, 2], F32, tag="tm1")
                    nc.vector.scalar_tensor_tensor(
                        tm0[:], a1, cs1[:, 1:2], blkslice(s0, t, blo, bhi),
                        AluOp.mult, AluOp.add)
                    nc.vector.scalar_tensor_tensor(
                        tm1[:], a0, cs1[:, 2:3], blkslice(s1, t, blo, bhi),
                        AluOp.mult, AluOp.add)
                    nc.vector.scalar_tensor_tensor(
                        blkslice(s0, t, blo, bhi), a0, cs1[:, 0:1], tm0[:],
                        AluOp.mult, AluOp.add)
                    nc.vector.scalar_tensor_tensor(
                        blkslice(s1, t, blo, bhi), a1, cs1[:, 3:4], tm1[:],
                        AluOp.mult, AluOp.add)
                # extract block-ends into compact bc tiles
                nc.vector.tensor_copy(
                    bc0[:, blo * 2:bhi * 2].rearrange("p (b r) -> p b r", r=2),
                    blkslice(s0, Q1 - 1, blo, bhi))
                nc.vector.tensor_copy(
                    bc1[:, blo * 2:bhi * 2].rearrange("p (b r) -> p b r", r=2),
                    blkslice(s1, Q1 - 1, blo, bhi))

            def stageAf():
                alo, ahi = 0, B2
                for t2 in range(1, Q2):
                    a0 = bslice(bc0, t2 - 1, alo, ahi)
                    a1 = bslice(bc1, t2 - 1, alo, ahi)
                    tb0 = sp.tile([P, ahi - alo, 2], F32, tag="tb0")
                    tb1 = sp.tile([P, ahi - alo, 2], F32, tag="tb1")
                    nc.vector.scalar_tensor_tensor(
                        tb0[:], a1, cbc[:, 1:2], bslice(bc0, t2, alo, ahi),
                        AluOp.mult, AluOp.add)
                    nc.vector.scalar_tensor_tensor(
                        tb1[:], a0, cbc[:, 2:3], bslice(bc1, t2, alo, ahi),
                        AluOp.mult, AluOp.add)
                    nc.vector.scalar_tensor_tensor(
                        bslice(bc0, t2, alo, ahi), a0, cbc[:, 0:1], tb0[:],
                        AluOp.mult, AluOp.add)
                    nc.vector.scalar_tensor_tensor(
                        bslice(bc1, t2, alo, ahi), a1, cbc[:, 3:4], tb1[:],
                        AluOp.mult, AluOp.add)

            def stageB():
                for a in range(1, B2):
                    pr0 = bc0[:, (a * Q2 - 1) * 2:(a * Q2 - 1) * 2 + 2]
                    pr1 = bc1[:, (a * Q2 - 1) * 2:(a * Q2 - 1) * 2 + 2]
                    cu0 = bc0[:, ((a + 1) * Q2 - 1) * 2:((a + 1) * Q2 - 1) * 2 + 2]
                    cu1 = bc1[:, ((a + 1) * Q2 - 1) * 2:((a + 1) * Q2 - 1) * 2 + 2]
                    tc0 = sp.tile([P, 2], F32, tag="tc0")
                    tc1 = sp.tile([P, 2], F32, tag="tc1")
                    nc.vector.scalar_tensor_tensor(
                        tc0[:], pr1, cbcA[:, 1:2], cu0, AluOp.mult, AluOp.add)
                    nc.vector.scalar_tensor_tensor(
                        tc1[:], pr0, cbcA[:, 2:3], cu1, AluOp.mult, AluOp.add)
                    nc.vector.scalar_tensor_tensor(
                        cu0, pr0, cbcA[:, 0:1], tc0[:], AluOp.mult, AluOp.add)
                    nc.vector.scalar_tensor_tensor(
                        cu1, pr1, cbcA[:, 3:4], tc1[:], AluOp.mult, AluOp.add)

            def stageCf():
                # fix bc of A-blocks (skip a=0; skip t2=Q2-1)
                alo, ahi = 1, B2
                na = ahi - alo
                for t2 in range(Q2 - 1):
                    vt0 = bslice(bc0, t2, alo, ahi)
                    vt1 = bslice(bc1, t2, alo, ahi)
                    pv0 = bslice(bc0, Q2 - 1, alo - 1, ahi - 1)
                    pv1 = bslice(bc1, Q2 - 1, alo - 1, ahi - 1)
                    tf0 = sp.tile([P, na, 2], F32, tag="tf0")
                    tf1 = sp.tile([P, na, 2], F32, tag="tf1")
                    co = 4 * t2
                    nc.vector.scalar_tensor_tensor(
                        tf0[:], pv1, fbc[:, co + 1:co + 2], vt0, AluOp.mult, AluOp.add)
                    nc.vector.scalar_tensor_tensor(
                        tf1[:], pv0, fbc[:, co + 2:co + 3], vt1, AluOp.mult, AluOp.add)
                    nc.vector.scalar_tensor_tensor(
                        vt0, pv0, fbc[:, co:co + 1], tf0[:], AluOp.mult, AluOp.add)
                    nc.vector.scalar_tensor_tensor(
                        vt1, pv1, fbc[:, co + 3:co + 4], tf1[:], AluOp.mult, AluOp.add)

            def l1fixf():
                blo, bhi = 1, B1
                nb = bhi - blo
                for t in range(Q1):
                    sv0 = blkslice(s0, t, blo, bhi)
                    sv1 = blkslice(s1, t, blo, bhi)
                    bp0 = bc0[:].rearrange("p (b r) -> p b r", r=2)[:, blo - 1:bhi - 1, :]
                    bp1 = bc1[:].rearrange("p (b r) -> p b r", r=2)[:, blo - 1:bhi - 1, :]
                    tg0 = sp.tile([P, nb, 2], F32, tag="tg0")
                    tg1 = sp.tile([P, nb, 2], F32, tag="tg1")
                    co = 4 * t
                    nc.vector.scalar_tensor_tensor(
                        tg0[:], bp1, fL1[:, co + 1:co + 2], sv0, AluOp.mult, AluOp.add)
                    nc.vector.scalar_tensor_tensor(
                        tg1[:], bp0, fL1[:, co + 2:co + 3], sv1, AluOp.mult, AluOp.add)
                    nc.vector.scalar_tensor_tensor(
                        sv0, bp0, fL1[:, co:co + 1], tg0[:], AluOp.mult, AluOp.add)
                    nc.vector.scalar_tensor_tensor(
                        sv1, bp1, fL1[:, co + 3:co + 4], tg1[:], AluOp.mult, AluOp.add)

            # shifted, f32r-rounded S' tiles: sh[k][ri][:, i] = S_{i-1}
            sh = {}
            for k in range(2):
                for ri in range(2):
                    sh[(k, ri)] = sp.tile([P, C], F32R, tag=f"sh{k}{ri}", name=f"sh{k}{ri}")

            def sprime_all():
                for k in range(2):
                    st = s0 if k == 0 else s1
                    for ri in range(2):
                        t_ = sh[(k, ri)]
                        nc.vector.memset(t_[:, 0:1].bitcast(F32), 0.0)
                        srcv = st[:].rearrange("p (c r) -> p c r", r=2)[:, 0:C - 1, ri]
                        nc.vector.tensor_copy(t_[:, 1:C], srcv)

            stage1f()
            stageAf()
            stageB()
            stageCf()
            l1fixf()
            sprime_all()

            # ---------- local (PE, chain-independent) ----------
            # all local lag-matmul groups run first so the PE is busy while
            # the DVE carry chain executes; results parked in SBUF by ACT.
            oloc = cp.tile([P, L], F32, tag="oloc")
            for cs in range(NSEG):
                for tau in range(T):
                    po = pso.tile([P, CSEG], F32, tag="pso")
                    for s in range(tau + 1):
                        nc.tensor.matmul(
                            po[:],
                            phi_sb[:, s * P:(s + 1) * P],
                            x3[:, cs * CSEG:(cs + 1) * CSEG, tau - s],
                            start=(s == 0), stop=(s == tau),
                        )
                    dst = oloc[:].rearrange("p (c t) -> p c t", t=T)[
                        :, cs * CSEG:(cs + 1) * CSEG, tau]
                    nc.scalar.copy(dst, po[:])

            # ---------- injection (PE, after chain) + assembly (DVE) ----------
            out_sb = cp.tile([P, L], F32, tag="out")
            AluOp2 = mybir.AluOpType
            for cs in range(NSEG):
                for tau in range(T):
                    pi = pso.tile([P, CSEG], F32, tag="pi")
                    mi = 0
                    for k in range(2):
                        for ri in range(2):
                            qidx = (tau * 2 + k) * 2 + ri
                            nc.tensor.matmul(
                                pi[:],
                                psi_sb[:, qidx * H:(qidx + 1) * H],
                                sh[(k, ri)][:, cs * CSEG:(cs + 1) * CSEG],
                                start=(mi == 0), stop=(mi == 3),
                            )
                            mi += 1
                    src_loc = oloc[:].rearrange("p (c t) -> p c t", t=T)[
                        :, cs * CSEG:(cs + 1) * CSEG, tau]
                    dst = out_sb[:].rearrange("p (c t) -> p c t", t=T)[
                        :, cs * CSEG:(cs + 1) * CSEG, tau]
                    nc.vector.tensor_tensor(dst, src_loc, pi[:], AluOp2.add)
                half = CSEG * T
                nc.sync.dma_start(out_d[:, cs * half:(cs + 1) * half],
                                  out_sb[:, cs * half:(cs + 1) * half])

    nc.compile()
    return nc


_NC_CACHE = None


def kernel(x, A_diag, G_diag, dt, B, C, D):
    global _NC_CACHE
    x = np.asarray(x, dtype=np.float32)
    A_diag = np.asarray(A_diag, np.float32)
    G_diag = np.asarray(G_diag, np.float32)
    dt = np.asarray(dt, np.float32)
    B = np.asarray(B, np.float32)
    C_ = np.asarray(C, np.float32)
    D = np.asarray(D, np.float32)

    wts = _host_weights(A_diag, G_diag, dt, B, C_, D)
    # lhsT orientation fixes:
    # local: lhsT[h_in, h_out] = Phi[h_out, h_in] -> PhiT already (H_in? ...)
    # PhiT built as Phi.T -> shape (H,H) with [h', h]; matmul lhsT partition dim
    # is contraction (h'), free is out partition (h). OK as built.
    xt = np.ascontiguousarray(x.transpose(0, 2, 1))      # (BSZ, H, L)
    xt = _tf32(xt)

    if _NC_CACHE is None:
        _NC_CACHE = _build_nc()
    nc = _NC_CACHE

    common = {"seed": wts["seed"], "kw": wts["kw"], "coef": wts["coef"]}
    in_maps = [dict(common, x=xt[b]) for b in range(BSZ)]
    res = bass_utils.run_bass_kernel_spmd(
        nc, in_maps, core_ids=list(range(BSZ)), trace=False)
    out = np.stack([res.results[b]["out"] for b in range(BSZ)], 0)  # (B, H, L)
    return np.ascontiguousarray(out.transpose(0, 2, 1))             # (B, L, H)

